# revision 6
# baseline (speedup 1.0000x reference)
"""LiquidNeuralNetwork Trainium2 kernel.

Math (per reference):
    xp   = einsum('bsi,hi->sbh', x, W_ih) + b_ih + b_hh          # [S, B, H]
    h_t  = tanh(xp_t + h_{t-1} @ W_hh.T)   for t in 0..S-1       # [B, H]
    out  = h_final @ W_fc.T + b_fc                               # [B, O]

Strategy: data-parallel over batch across 8 cores (8 examples/core).
Everything on-chip is kept in the TRANSPOSED layout (feature dim on
partitions, batch on the free dim), so the sequential recurrence needs no
per-step transpose:

    zT_m = sum_k W_hhT[k,m].T @ hT_k        (PE: stationary [128,128] bf16
                                             weight tiles -> FWL fast load,
                                             streaming hT [128,8])
    hT_m = tanh(zT_m + xpT_t[m])            (DVE add + ACT tanh)

Phase 1 computes xpT = W_ih @ x.T as a big GEMM (x transposed on-chip via
PE-transpose), staged to DRAM as [H, S*BL] f32.  Phase 2 runs the 512-step
recurrence (bf16 matmuls, f32 accumulate).  Phase 3 is the small FC.
"""

import numpy as np
import ml_dtypes

import concourse.bass as bass
from concourse import bacc
import concourse.mybir as mybir
import concourse.tile as tile
from concourse.bass_utils import run_bass_kernel_spmd
from concourse.masks import make_identity

B, S, I, H, O = 64, 512, 512, 1024, 512
NCORES = 8
BL = B // NCORES  # batch per core

F32 = mybir.dt.float32
BF16 = mybir.dt.bfloat16

KH = H // 128   # 8 k-tiles over hidden dim
KI = I // 128   # 4 k-tiles over input dim
NSB = (S * BL) // 512  # 8 column-chunks of 512 (s,b) pairs in phase 1
GSTEPS = 16            # recurrence steps per xpT DMA group
AF = mybir.ActivationFunctionType


def build_nc(steps: int = S) -> bass.Bass:
    nc = bacc.Bacc()

    x = nc.dram_tensor("x", [BL, S, I], F32, kind="ExternalInput")
    w_ihT = nc.dram_tensor("w_ihT", [I, H], BF16, kind="ExternalInput")
    w_hhT = nc.dram_tensor("w_hhT", [H, H], BF16, kind="ExternalInput")
    w_fcT = nc.dram_tensor("w_fcT", [H, O], BF16, kind="ExternalInput")
    bias_h = nc.dram_tensor("bias_h", [H, 1], F32, kind="ExternalInput")
    b_fc8 = nc.dram_tensor("b_fc8", [BL, O], F32, kind="ExternalInput")
    out = nc.dram_tensor("out", [BL, O], F32, kind="ExternalOutput")
    xpT = nc.dram_tensor("xpT_stage", [H, S * BL], F32)

    xr = x[:].rearrange("b s i -> s b i")  # [S, BL, I]

    with tile.TileContext(nc) as tc:
        with tc.tile_pool(name="consts", bufs=1) as consts:
            ident = consts.tile([128, 128], F32, tag="ident")
            make_identity(nc, ident[:])

            wih_sb = []
            for k in range(KI):
                t_ = consts.tile([128, H], BF16, tag=f"wih{k}")
                nc.sync.dma_start(out=t_[:], in_=w_ihT[k * 128:(k + 1) * 128, :])
                wih_sb.append(t_)
            whh_sb = []
            for k in range(KH):
                t_ = consts.tile([128, H], BF16, tag=f"whh{k}")
                nc.sync.dma_start(out=t_[:], in_=w_hhT[k * 128:(k + 1) * 128, :])
                whh_sb.append(t_)
            wfc_sb = []
            for k in range(KH):
                t_ = consts.tile([128, O], BF16, tag=f"wfc{k}")
                nc.sync.dma_start(out=t_[:], in_=w_fcT[k * 128:(k + 1) * 128, :])
                wfc_sb.append(t_)
            bias_sb = []
            for m in range(KH):
                t_ = consts.tile([128, 1], F32, tag=f"bias{m}")
                nc.sync.dma_start(out=t_[:], in_=bias_h[m * 128:(m + 1) * 128, :])
                bias_sb.append(t_)
            bfc_sb = consts.tile([BL, O], F32, tag="bfc")
            nc.sync.dma_start(out=bfc_sb[:], in_=b_fc8[:])

            # ---------------- Phase 1: xpT = W_ih @ x.T + bias ----------------
            with (
                tc.tile_pool(name="natx", bufs=4) as natx_p,
                tc.tile_pool(name="xt", bufs=2) as xt_p,
                tc.tile_pool(name="xpout", bufs=4) as xpout_p,
                tc.tile_pool(name="ph1pst", bufs=4, space="PSUM") as ph1pst,
                tc.tile_pool(name="ph1psm", bufs=4, space="PSUM") as ph1psm,
            ):
                for n in range(NSB):  # 512-wide (s,b) chunks
                    xt_tiles = [xt_p.tile([128, 512], BF16, tag=f"xt{k}",
                                          name=f"xt{k}")
                                for k in range(KI)]
                    for rt in range(4):  # 128-row subchunks (16 s x 8 b)
                        nat = natx_p.tile([128, I], F32, tag="nat")
                        s0 = n * 64 + rt * 16
                        nc.sync.dma_start(out=nat[:], in_=xr[s0:s0 + 16, :, :])
                        for k in range(KI):
                            pst = ph1pst.tile([128, 128], F32, tag="pst")
                            nc.tensor.transpose(
                                pst[:], nat[:, k * 128:(k + 1) * 128], ident[:])
                            nc.vector.tensor_copy(
                                xt_tiles[k][:, rt * 128:(rt + 1) * 128], pst[:])
                    for m in range(KH):
                        ps = ph1psm.tile([128, 512], F32, tag="ps")
                        for k in range(KI):
                            nc.tensor.matmul(
                                ps[:],
                                wih_sb[k][:, m * 128:(m + 1) * 128],
                                xt_tiles[k][:],
                                start=(k == 0), stop=(k == KI - 1))
                        xo = xpout_p.tile([128, 512], F32, tag="xo")
                        nc.scalar.activation(
                            xo[:], ps[:], AF.Identity, bias=bias_sb[m][:])
                        nc.sync.dma_start(
                            out=xpT[m * 128:(m + 1) * 128,
                                    n * 512:(n + 1) * 512],
                            in_=xo[:])

            # ---------------- Phase 2: recurrence ----------------
            with (
                tc.tile_pool(name="xpg", bufs=2) as xpg_p,
                tc.tile_pool(name="ht", bufs=3) as ht_p,
                tc.tile_pool(name="tmp", bufs=2) as tmp_p,
                tc.tile_pool(name="recps", bufs=KH, space="PSUM") as recps,
            ):
                h_prev = None
                xpg = None
                for t in range(steps):
                    g, o = divmod(t, GSTEPS)
                    if o == 0:
                        xpg = [xpg_p.tile([128, GSTEPS * BL], F32,
                                          tag=f"xpg{m}", name=f"xpg{m}")
                               for m in range(KH)]
                        for m in range(KH):
                            nc.sync.dma_start(
                                out=xpg[m][:],
                                in_=xpT[m * 128:(m + 1) * 128,
                                        g * GSTEPS * BL:(g + 1) * GSTEPS * BL])
                    h_cur = [ht_p.tile([128, BL], BF16, tag=f"ht{m}",
                                       name=f"ht{m}")
                             for m in range(KH)]
                    for m in range(KH):
                        xp_sl = xpg[m][:, o * BL:(o + 1) * BL]
                        if t == 0:
                            nc.scalar.activation(h_cur[m][:], xp_sl, AF.Tanh)
                            continue
                        ps = recps.tile([128, BL], F32, tag="ps", name=f"ps{m}")
                        for k in range(KH):
                            nc.tensor.matmul(
                                ps[:],
                                whh_sb[k][:, m * 128:(m + 1) * 128],
                                h_prev[k][:],
                                start=(k == 0), stop=(k == KH - 1))
                        tmp = tmp_p.tile([128, BL], F32, tag=f"tmp{m}", name=f"tmp{m}")
                        nc.vector.tensor_add(tmp[:], ps[:], xp_sl)
                        nc.scalar.activation(h_cur[m][:], tmp[:], AF.Tanh)
                    h_prev = h_cur

            # ---------------- Phase 3: FC ----------------
            with (
                tc.tile_pool(name="fco", bufs=1) as fco_p,
                tc.tile_pool(name="fcps", bufs=1, space="PSUM") as fcps,
            ):
                ps = fcps.tile([BL, O], F32, tag="fcps")
                for k in range(KH):
                    nc.tensor.matmul(ps[:], h_prev[k][:], wfc_sb[k][:],
                                     start=(k == 0), stop=(k == KH - 1))
                ob = fco_p.tile([BL, O], F32, tag="ob")
                nc.vector.tensor_add(ob[:], ps[:], bfc_sb[:])
                nc.sync.dma_start(out=out[:], in_=ob[:])

    nc.compile()
    return nc


def make_in_maps(x, W_ih, W_hh, b_ih, b_hh, W_fc, b_fc):
    x = np.asarray(x, np.float32)
    w_ihT = np.ascontiguousarray(np.asarray(W_ih, np.float32).T).astype(
        ml_dtypes.bfloat16)
    w_hhT = np.ascontiguousarray(np.asarray(W_hh, np.float32).T).astype(
        ml_dtypes.bfloat16)
    w_fcT = np.ascontiguousarray(np.asarray(W_fc, np.float32).T).astype(
        ml_dtypes.bfloat16)
    bias_h = (np.asarray(b_ih, np.float32)
              + np.asarray(b_hh, np.float32)).reshape(H, 1)
    b_fc8 = np.tile(np.asarray(b_fc, np.float32), (BL, 1))
    return [
        {"x": np.ascontiguousarray(x[c * BL:(c + 1) * BL]),
         "w_ihT": w_ihT, "w_hhT": w_hhT, "w_fcT": w_fcT,
         "bias_h": bias_h, "b_fc8": b_fc8}
        for c in range(NCORES)
    ]


def kernel(x, W_ih, W_hh, b_ih, b_hh, W_fc, b_fc):
    nc = build_nc()
    in_maps = make_in_maps(x, W_ih, W_hh, b_ih, b_hh, W_fc, b_fc)
    res = run_bass_kernel_spmd(nc, in_maps, list(range(NCORES)))
    return np.concatenate(
        [np.asarray(res.results[c]["out"], np.float32) for c in range(NCORES)],
        axis=0)
